# revision 2
# baseline (speedup 1.0000x reference)
"""Trainium2 Bass kernel for nn_Backbone_Net (ResNet-ish backbone + FC + projection).

Data-parallel over 8 NeuronCores: 256 samples/core, processed in 16 chunks of 16.
Layout: channel-major activations [C_part, batch, H, W]. BN folded into weights
on host. conv1 via host im2col (bf16, K=147). 3x3 convs = 9 shifted f32r matmuls
accumulating in PSUM over zero-padded canvases. FC in bf16 with streamed weights.
Final PE transpose + on-device stereographic projection.
"""
import numpy as np
import ml_dtypes
from contextlib import ExitStack

import concourse.bass as bass
from concourse import bacc
import concourse.mybir as mybir
import concourse.tile as tile
from concourse.bass_utils import run_bass_kernel_spmd
from concourse.masks import make_identity

F32 = mybir.dt.float32
F32R = mybir.dt.float32r
BF16 = mybir.dt.bfloat16
AF = mybir.ActivationFunctionType
ALU = mybir.AluOpType
BF = ml_dtypes.bfloat16

NCORES = 8
B_TOTAL = 2048
BPC = B_TOTAL // NCORES      # 256 samples per core
BC = 16                      # chunk size (samples)
NCHUNK = BPC // BC           # 16
BN_EPS = 1e-5

# bias pack column registry: name -> (col_start, ncols, rows)
BIAS_SPECS = (
    [("bn1", 64), ("l1b0_b1", 64), ("l1b0_b2", 64), ("l1b0_b3", 256)]
    + [(f"l1b{i}_b{j}", c) for i in (1, 2) for j, c in ((1, 64), (2, 64), (3, 256))]
    + [(f"l2b{i}_b{j}", c) for i in range(4) for j, c in ((1, 128), (2, 128), (3, 512))]
    + [("fc_b", 512), ("fc1_b", 256)]
)


def _bias_cols():
    cols = {}
    c = 0
    for name, ch in BIAS_SPECS:
        n = max(1, ch // 128)
        cols[name] = (c, n, min(ch, 128))
        c += n
    return cols, c


BCOLS, NBIAS = _bias_cols()


def _fold(w, bn):
    s = bn["g"] / np.sqrt(bn["v"] + BN_EPS)
    return (w * s[:, None, None, None]).astype(np.float32), (bn["b"] - bn["m"] * s).astype(np.float32)


def _np(t):
    if isinstance(t, dict):
        return {k: _np(v) for k, v in t.items()}
    if isinstance(t, list):
        return [_np(v) for v in t]
    return np.asarray(t, dtype=np.float32)


def _prep_host(x, params):
    """Fold BN, build matmul-ready weight arrays + im2col input. Returns
    (shared weight map, per-core xcol slices)."""
    p = _np(params)
    x = np.asarray(x, dtype=np.float32)
    w = {}

    # --- conv1: 7x7/2 pad3, im2col K=147 ---
    wc, bc1 = _fold(p["conv1_w"], p["bn1"])                     # [64,3,7,7]
    wcol = wc.transpose(1, 2, 3, 0).reshape(147, 64)             # rows (c,ky,kx)
    w["c1wa"] = wcol[:128].astype(BF)
    w["c1wb"] = wcol[128:].astype(BF)

    biases = np.zeros((128, NBIAS), np.float32)

    def put_bias(name, vec):
        c0, n, rows = BCOLS[name]
        if n == 1:
            biases[:rows, c0] = vec
        else:
            biases[:, c0:c0 + n] = vec.reshape(n, 128).T

    put_bias("bn1", bc1)

    # --- layer1 ---
    for i, blk in enumerate(p["layer1"]):
        w1, b1 = _fold(blk["w1"], blk["b1"])    # [64, cin,1,1]
        w2, b2 = _fold(blk["w2"], blk["b2"])    # [64,64,3,3]
        w3, b3 = _fold(blk["w3"], blk["b3"])    # [256,64,1,1]
        w1l = w1[:, :, 0, 0].T                  # [cin, 64]
        if i == 0:
            dw, bd = _fold(blk["dw"], blk["db"])  # [256,64,1,1]
            w["l1b0_w1"] = w1l.astype(BF)         # [64,64] (reads bf16 m0)
            w["l1b0_dw"] = dw[:, :, 0, 0].T.reshape(64, 2, 128).astype(BF)
            b3 = b3 + bd
        else:
            w[f"l1b{i}_w1"] = w1l.reshape(2, 128, 64).transpose(1, 0, 2).copy()  # [128,2,64]
        w[f"l1b{i}_w2"] = w2.transpose(1, 2, 3, 0).reshape(64, 9, 64).copy()
        w[f"l1b{i}_w3"] = w3[:, :, 0, 0].T.reshape(64, 2, 128).copy()
        put_bias(f"l1b{i}_b1", b1)
        put_bias(f"l1b{i}_b2", b2)
        put_bias(f"l1b{i}_b3", b3)

    # --- layer2 ---
    for i, blk in enumerate(p["layer2"]):
        w1, b1 = _fold(blk["w1"], blk["b1"])    # [128, cin,1,1]
        w2, b2 = _fold(blk["w2"], blk["b2"])    # [128,128,3,3]
        w3, b3 = _fold(blk["w3"], blk["b3"])    # [512,128,1,1]
        w1l = w1[:, :, 0, 0].T                  # [cin,128]
        kg = w1l.shape[0] // 128
        w[f"l2b{i}_w1"] = w1l.reshape(kg, 128, 128).transpose(1, 0, 2).copy()  # [128,kg,128]
        w[f"l2b{i}_w2"] = w2.transpose(1, 2, 3, 0).reshape(128, 9, 128).copy()
        w[f"l2b{i}_w3"] = w3[:, :, 0, 0].T.reshape(128, 4, 128).copy()
        if i == 0:
            dw, bd = _fold(blk["dw"], blk["db"])  # [512,256,1,1]
            w["l2b0_dw"] = dw[:, :, 0, 0].T.reshape(2, 128, 4, 128).transpose(1, 0, 2, 3).copy()
            b3 = b3 + bd
        put_bias(f"l2b{i}_b1", b1)
        put_bias(f"l2b{i}_b2", b2)
        put_bias(f"l2b{i}_b3", b3)

    # --- fc layers (bf16) ---
    put_bias("fc_b", p["fc_b"])
    put_bias("fc1_b", p["fc1_b"])
    # FCW[m,g,p,s,mo] = fc_w[m*128+mo, (g*128+p)*16+s]
    w["fcw"] = p["fc_w"].reshape(4, 128, 4, 128, 16).transpose(0, 2, 3, 4, 1).astype(BF)
    # FC1W[p,k,m2,mo] = fc1_w[m2*128+mo, k*128+p]
    w["fc1w"] = p["fc1_w"].reshape(2, 128, 4, 128).transpose(3, 2, 0, 1).astype(BF)
    w["biases"] = biases

    # --- input im2col (bf16): xcol[(c,ky,kx), b, (y,x)] ---
    xp = np.zeros((B_TOTAL, 3, 38, 38), np.float32)
    xp[:, :, 3:35, 3:35] = x
    win = np.lib.stride_tricks.sliding_window_view(xp, (7, 7), axis=(2, 3))
    win = win[:, :, ::2, ::2]                                   # [B,3,16,16,7,7]
    xcol = win.transpose(1, 4, 5, 0, 2, 3).reshape(147, B_TOTAL, 256).astype(BF)
    xcols = [np.ascontiguousarray(xcol[:, c * BPC:(c + 1) * BPC, :]) for c in range(NCORES)]
    return w, xcols


def _build_program():
    nc = bacc.Bacc("TRN2", target_bir_lowering=False, debug=False)

    # ---- DRAM tensors ----
    d = {}
    def din(name, shape, dt):
        d[name] = nc.dram_tensor(name, shape, dt, kind="ExternalInput")
        return d[name]

    din("xcol", [147, BPC, 256], BF16)
    din("c1wa", [128, 64], BF16)
    din("c1wb", [19, 64], BF16)
    din("biases", [128, NBIAS], F32)
    din("l1b0_w1", [64, 64], BF16)
    din("l1b0_dw", [64, 2, 128], BF16)
    for i in range(3):
        din(f"l1b{i}_w2", [64, 9, 64], F32R)
        din(f"l1b{i}_w3", [64, 2, 128], F32R)
        if i > 0:
            din(f"l1b{i}_w1", [128, 2, 64], F32R)
    for i in range(4):
        kg = 2 if i == 0 else 4
        din(f"l2b{i}_w1", [128, kg, 128], F32R)
        din(f"l2b{i}_w2", [128, 9, 128], F32R)
        din(f"l2b{i}_w3", [128, 4, 128], F32R)
    din("l2b0_dw", [128, 2, 4, 128], F32R)
    din("fcw", [4, 4, 128, 16, 128], BF16)
    din("fc1w", [128, 4, 2, 128], BF16)
    out = nc.dram_tensor("out", [BPC, 257], F32, kind="ExternalOutput")

    with ExitStack() as ctx:
        tc = ctx.enter_context(tile.TileContext(nc))
        wgt = ctx.enter_context(tc.tile_pool(name="wgt", bufs=1))
        psum = ctx.enter_context(tc.tile_pool(name="ps", bufs=6, space="PSUM"))

        # ---- resident weights ----
        s = {}
        for name, shape, dt in [
            ("c1wa", [128, 64], BF16), ("c1wb", [19, 64], BF16),
            ("biases", [128, NBIAS], F32),
            ("l1b0_w1", [64, 64], BF16), ("l1b0_dw", [64, 2, 128], BF16),
            ("l1b0_w2", [64, 9, 64], F32R), ("l1b0_w3", [64, 2, 128], F32R),
            ("l1b1_w1", [128, 2, 64], F32R), ("l1b1_w2", [64, 9, 64], F32R), ("l1b1_w3", [64, 2, 128], F32R),
            ("l1b2_w1", [128, 2, 64], F32R), ("l1b2_w2", [64, 9, 64], F32R), ("l1b2_w3", [64, 2, 128], F32R),
            ("l2b0_w1", [128, 2, 128], F32R), ("l2b0_w2", [128, 9, 128], F32R),
            ("l2b0_w3", [128, 4, 128], F32R), ("l2b0_dw", [128, 2, 4, 128], F32R),
            ("l2b1_w1", [128, 4, 128], F32R), ("l2b1_w2", [128, 9, 128], F32R), ("l2b1_w3", [128, 4, 128], F32R),
            ("l2b2_w1", [128, 4, 128], F32R), ("l2b2_w2", [128, 9, 128], F32R), ("l2b2_w3", [128, 4, 128], F32R),
            ("l2b3_w1", [128, 4, 128], F32R), ("l2b3_w2", [128, 9, 128], F32R), ("l2b3_w3", [128, 4, 128], F32R),
            ("fc1w", [128, 4, 2, 128], BF16),
        ]:
            t = wgt.tile(shape, dt, tag=name)
            nc.sync.dma_start(out=t, in_=d[name].ap())
            s[name] = t

        def bias_ap(name, g=0):
            c0, n, rows = BCOLS[name]
            assert g < n
            return s["biases"][0:rows, c0 + g:c0 + g + 1]

        # persistent accumulator for FC inputs: A5[p, m, s, b] bf16
        A5 = wgt.tile([128, 4, 16, BPC], BF16, tag="A5")
        A5v = A5.rearrange("p m s b -> p m b s")

        ident = wgt.tile([128, 128], F32, tag="ident")
        make_identity(nc, ident)

        # ---- persistent padded canvases (borders zeroed once) ----
        c1p = wgt.tile([64, BC, 17, 17], BF16, tag="c1p")
        nc.gpsimd.memset(c1p[:, :, 0, :], 0.0)
        nc.gpsimd.memset(c1p[:, :, 1:17, 0], 0.0)
        l1mid = wgt.tile([64, BC, 10, 10], F32R, tag="l1mid")
        l2b0mid = wgt.tile([128, BC, 10, 10], F32R, tag="l2b0mid")
        for cv in (l1mid, l2b0mid):
            nc.gpsimd.memset(cv[:, :, 0, :].bitcast(F32), 0.0)
            nc.gpsimd.memset(cv[:, :, 9, :].bitcast(F32), 0.0)
            nc.gpsimd.memset(cv[:, :, 1:9, 0].bitcast(F32), 0.0)
            nc.gpsimd.memset(cv[:, :, 1:9, 9].bitcast(F32), 0.0)
        l2mids = []
        for cvi in range(2):
            cv = wgt.tile([128, BC, 6, 6], F32R, tag=f"l2mid{cvi}")
            nc.gpsimd.memset(cv[:, :, 0, :].bitcast(F32), 0.0)
            nc.gpsimd.memset(cv[:, :, 5, :].bitcast(F32), 0.0)
            nc.gpsimd.memset(cv[:, :, 1:5, 0].bitcast(F32), 0.0)
            nc.gpsimd.memset(cv[:, :, 1:5, 5].bitcast(F32), 0.0)
            l2mids.append(cv)

        # ---- chunk-phase pools ----
        with ExitStack() as cctx:
            xap = cctx.enter_context(tc.tile_pool(name="xap", bufs=2))
            xbp = cctx.enter_context(tc.tile_pool(name="xbp", bufs=2))
            mpool = cctx.enter_context(tc.tile_pool(name="mpool", bufs=2))
            apool = cctx.enter_context(tc.tile_pool(name="apool", bufs=3))

            for ci in range(NCHUNK):
                b0 = ci * BC
                N8 = BC * 64     # 1024 (8x8 spatial)
                # -- conv1 + maxpool --
                for h in range(2):  # half-chunks of 8 samples
                    xa = xap.tile([128, 8 * 256], BF16, tag="xa")
                    xb = xbp.tile([19, 8 * 256], BF16, tag="xb")
                    nc.sync.dma_start(out=xa, in_=d["xcol"].ap()[0:128, b0 + 8 * h:b0 + 8 * h + 8, :])
                    nc.sync.dma_start(out=xb, in_=d["xcol"].ap()[128:147, b0 + 8 * h:b0 + 8 * h + 8, :])
                    for n in range(4):  # 512-wide tiles, 2 samples each
                        ps = psum.tile([64, 512], F32, tag="ps")
                        nc.tensor.matmul(ps, s["c1wa"], xa[:, n * 512:(n + 1) * 512], start=True, stop=False)
                        nc.tensor.matmul(ps, s["c1wb"], xb[:, n * 512:(n + 1) * 512], start=False, stop=True)
                        smp = 8 * h + 2 * n
                        nc.vector.tensor_scalar(
                            out=c1p[:, smp:smp + 2, 1:17, 1:17],
                            in0=ps.rearrange("p (b y x) -> p b y x", b=2, y=16),
                            scalar1=bias_ap("bn1"), scalar2=0.0, op0=ALU.add, op1=ALU.max)
                # separable 3x3/2 maxpool (pad 0 at top/left; values >= 0)
                tcol = mpool.tile([64, BC, 17, 8], BF16, tag="tcol")
                nc.vector.tensor_max(out=tcol, in0=c1p[:, :, :, 0:15:2], in1=c1p[:, :, :, 1:16:2])
                nc.vector.tensor_max(out=tcol, in0=tcol, in1=c1p[:, :, :, 2:17:2])
                m0 = mpool.tile([64, BC, 8, 8], BF16, tag="m0")
                nc.vector.tensor_max(out=m0, in0=tcol[:, :, 0:15:2, :], in1=tcol[:, :, 1:16:2, :])
                nc.vector.tensor_max(out=m0, in0=m0, in1=tcol[:, :, 2:17:2, :])
                m0f = m0.rearrange("p b y x -> p (b y x)")

                # -- layer1 --
                a_prev = None
                for blk in range(3):
                    nm = f"l1b{blk}"
                    # mid1 = relu(w1 @ in)
                    for n in range(2):
                        ns = slice(n * 512, (n + 1) * 512)
                        ps = psum.tile([64, 512], F32, tag="ps")
                        if blk == 0:
                            nc.tensor.matmul(ps, s["l1b0_w1"], m0f[:, ns], start=True, stop=True)
                        else:
                            for g in range(2):
                                nc.tensor.matmul(ps, s[f"{nm}_w1"][:, g, :], a_prev[:, g, ns],
                                                 start=(g == 0), stop=(g == 1))
                        nc.scalar.activation(
                            out=l1mid[:, 8 * n:8 * n + 8, 1:9, 1:9],
                            in_=ps.rearrange("p (b y x) -> p b y x", b=8, y=8),
                            func=AF.Relu, bias=bias_ap(f"{nm}_b1"), scale=1.0)
                    # mid2 = relu(conv3x3(mid1))
                    mid2 = mpool.tile([64, BC, 8, 8], F32R, tag="mid2l1")
                    for bh in range(2):
                        ps = psum.tile([64, 512], F32, tag="ps")
                        for t in range(9):
                            ky, kx = t // 3, t % 3
                            nc.tensor.matmul(ps, s[f"{nm}_w2"][:, t, :],
                                             l1mid[:, 8 * bh:8 * bh + 8, ky:ky + 8, kx:kx + 8],
                                             start=(t == 0), stop=(t == 8))
                        nc.scalar.activation(
                            out=mid2[:, 8 * bh:8 * bh + 8, :, :],
                            in_=ps.rearrange("p (b y x) -> p b y x", b=8, y=8),
                            func=AF.Relu, bias=bias_ap(f"{nm}_b2"), scale=1.0)
                    mid2f = mid2.rearrange("p b y x -> p (b y x)")
                    # out = relu(w3 @ mid2 (+ dw @ m0 | + identity))
                    a_new = apool.tile([128, 2, N8], F32R, tag="l1a")
                    for g in range(2):
                        for n in range(2):
                            ns = slice(n * 512, (n + 1) * 512)
                            ps = psum.tile([128, 512], F32, tag="ps")
                            nc.tensor.matmul(ps, s[f"{nm}_w3"][:, g, :], mid2f[:, ns],
                                             start=True, stop=(blk != 0))
                            if blk == 0:
                                nc.tensor.matmul(ps, s["l1b0_dw"][:, g, :], m0f[:, ns],
                                                 start=False, stop=True)
                                nc.scalar.activation(out=a_new[:, g, ns], in_=ps, func=AF.Relu,
                                                     bias=bias_ap(f"{nm}_b3", g), scale=1.0)
                            else:
                                nc.vector.scalar_tensor_tensor(
                                    out=a_new[:, g, ns], in0=ps,
                                    scalar=bias_ap(f"{nm}_b3", g),
                                    in1=a_prev[:, g, ns],
                                    op0=ALU.add, op1=ALU.add)
                    if blk != 0:
                        af = a_new.rearrange("p g n -> p (g n)")
                        nc.scalar.activation(out=af, in_=af, func=AF.Relu)
                    a_prev = a_new

                # -- layer2 --
                N4 = BC * 16    # 256 (4x4 spatial)
                a3 = a_prev     # [128, 2, 1024]
                for blk in range(4):
                    nm = f"l2b{blk}"
                    kg = 2 if blk == 0 else 4
                    if blk == 0:
                        canvas = l2b0mid
                        ain = a3
                        for n in range(2):
                            ns = slice(n * 512, (n + 1) * 512)
                            ps = psum.tile([128, 512], F32, tag="ps")
                            for g in range(2):
                                nc.tensor.matmul(ps, s[f"{nm}_w1"][:, g, :], ain[:, g, ns],
                                                 start=(g == 0), stop=(g == 1))
                            nc.scalar.activation(
                                out=canvas[:, 8 * n:8 * n + 8, 1:9, 1:9],
                                in_=ps.rearrange("p (b y x) -> p b y x", b=8, y=8),
                                func=AF.Relu, bias=bias_ap(f"{nm}_b1"), scale=1.0)
                    else:
                        canvas = l2mids[blk % 2]
                        ain = a_prev  # [128, 4, 256]
                        ps = psum.tile([128, 256], F32, tag="ps")
                        for g in range(4):
                            nc.tensor.matmul(ps, s[f"{nm}_w1"][:, g, :], ain[:, g, :],
                                             start=(g == 0), stop=(g == 3))
                        nc.scalar.activation(
                            out=canvas[:, :, 1:5, 1:5],
                            in_=ps.rearrange("p (b y x) -> p b y x", b=BC, y=4),
                            func=AF.Relu, bias=bias_ap(f"{nm}_b1"), scale=1.0)
                    # mid2 = relu(conv3x3(canvas)), stride 2 for blk 0
                    mid2 = mpool.tile([128, BC, 4, 4], F32R, tag="mid2l2")
                    ps = psum.tile([128, 256], F32, tag="ps")
                    for t in range(9):
                        ky, kx = t // 3, t % 3
                        if blk == 0:
                            rhs = canvas[:, :, ky:ky + 8:2, kx:kx + 8:2]
                        else:
                            rhs = canvas[:, :, ky:ky + 4, kx:kx + 4]
                        nc.tensor.matmul(ps, s[f"{nm}_w2"][:, t, :], rhs,
                                         start=(t == 0), stop=(t == 8))
                    nc.vector.tensor_scalar(
                        out=mid2, in0=ps.rearrange("p (b y x) -> p b y x", b=BC, y=4),
                        scalar1=bias_ap(f"{nm}_b2"), scalar2=0.0, op0=ALU.add, op1=ALU.max)
                    mid2f = mid2.rearrange("p b y x -> p (b y x)")
                    # out = relu(w3 @ mid2 (+ dw @ a3 | + identity))
                    last = blk == 3
                    if not last:
                        a_new = apool.tile([128, 4, N4], F32R, tag="l2a")
                    a3s = a3.rearrange("p g (b y x) -> p g b y x", b=BC, y=8)
                    for m in range(4):
                        ps = psum.tile([128, 256], F32, tag="ps")
                        nc.tensor.matmul(ps, s[f"{nm}_w3"][:, m, :], mid2f,
                                         start=True, stop=(blk != 0))
                        if blk == 0:
                            for g in range(2):
                                nc.tensor.matmul(ps, s["l2b0_dw"][:, g, m, :],
                                                 a3s[:, g, :, 0:8:2, 0:8:2],
                                                 start=False, stop=(g == 1))
                            nc.scalar.activation(out=a_new[:, m, :], in_=ps, func=AF.Relu,
                                                 bias=bias_ap(f"{nm}_b3", m), scale=1.0)
                        elif not last:
                            nc.vector.scalar_tensor_tensor(
                                out=a_new[:, m, :], in0=ps,
                                scalar=bias_ap(f"{nm}_b3", m),
                                in1=a_prev[:, m, :],
                                op0=ALU.add, op1=ALU.add)
                        else:
                            nc.vector.scalar_tensor_tensor(
                                out=A5v[:, m, b0:b0 + BC, :],
                                in0=ps.rearrange("p (b s) -> p b s", b=BC),
                                scalar=bias_ap(f"{nm}_b3", m),
                                in1=a_prev[:, m, :].rearrange("p (b s) -> p b s", b=BC),
                                op0=ALU.add, op1=ALU.add)
                    if blk == 0:
                        pass
                    elif not last:
                        af = a_new.rearrange("p m n -> p (m n)")
                        nc.scalar.activation(out=af, in_=af, func=AF.Relu)
                    else:
                        av = A5v[:, :, b0:b0 + BC, :]
                        nc.scalar.activation(out=av, in_=av, func=AF.Relu)
                    a_prev = a_new if not last else None

        # ---- FC phase ----
        with ExitStack() as fctx:
            fcp = fctx.enter_context(tc.tile_pool(name="fcp", bufs=2))
            fc1 = fctx.enter_context(tc.tile_pool(name="fc1", bufs=1))
            z1 = fc1.tile([128, 4, BPC], BF16, tag="z1")
            for m in range(4):
                fcw_t = fcp.tile([128, 4, 16, 128], BF16, tag="fcw")
                nc.sync.dma_start(out=fcw_t, in_=d["fcw"].ap()[m].rearrange("g p s mo -> p g s mo"))
                ps = psum.tile([128, BPC], F32, tag="ps")
                for g in range(4):
                    for si in range(16):
                        nc.tensor.matmul(ps, fcw_t[:, g, si, :], A5[:, g, si, :],
                                         start=(g == 0 and si == 0), stop=(g == 3 and si == 15))
                nc.scalar.activation(out=z1[:, m, :], in_=ps, func=AF.Relu,
                                     bias=bias_ap("fc_b", m), scale=1.0)
            y2 = fc1.tile([128, 2, BPC], F32, tag="y2")
            for m2 in range(2):
                ps = psum.tile([128, BPC], F32, tag="ps")
                for k in range(4):
                    nc.tensor.matmul(ps, s["fc1w"][:, k, m2, :], z1[:, k, :],
                                     start=(k == 0), stop=(k == 3))
                nc.scalar.activation(out=y2[:, m2, :], in_=ps, func=AF.Relu,
                                     bias=bias_ap("fc1_b", m2), scale=1.0)
            # transpose y2 -> y2t[b_part, bh, ch]
            y2t = fc1.tile([128, 2, 257], F32, tag="y2t")
            for g in range(2):
                for bh in range(2):
                    ps = psum.tile([128, 128], F32, tag="ps")
                    nc.tensor.transpose(ps, y2[:, g, bh * 128:(bh + 1) * 128], ident)
                    nc.vector.tensor_copy(out=y2t[:, bh, g * 128:(g + 1) * 128], in_=ps)
            # stereographic projection (per-sample, samples on partitions)
            sq = fc1.tile([128, 256], F32, tag="sq")
            ss = fc1.tile([128, 2], F32, tag="ss")
            rec = fc1.tile([128, 2], F32, tag="rec")
            s_t = fc1.tile([128, 2], F32, tag="s_t")
            oms = fc1.tile([128, 2], F32, tag="oms")
            for bh in range(2):
                nc.scalar.activation(out=sq, in_=y2t[:, bh, 0:256], func=AF.Square,
                                     accum_out=ss[:, bh:bh + 1])
            nc.vector.tensor_scalar(out=rec, in0=ss, scalar1=1.0, scalar2=None, op0=ALU.add)
            nc.vector.reciprocal(out=rec, in_=rec)
            nc.vector.scalar_tensor_tensor(out=s_t, in0=ss, scalar=-1.0, in1=rec,
                                           op0=ALU.add, op1=ALU.mult)
            nc.vector.tensor_scalar(out=oms, in0=s_t, scalar1=-1.0, scalar2=1.0,
                                    op0=ALU.mult, op1=ALU.add)
            for bh in range(2):
                nc.vector.tensor_scalar(out=y2t[:, bh, 0:256], in0=y2t[:, bh, 0:256],
                                        scalar1=oms[:, bh:bh + 1], scalar2=None, op0=ALU.mult)
                nc.vector.tensor_copy(out=y2t[:, bh, 256:257], in_=s_t[:, bh:bh + 1])
            nc.sync.dma_start(out=out.ap().rearrange("(bh p) c -> p bh c", p=128), in_=y2t)

    nc.compile()
    return nc


_NC = None


def _program():
    global _NC
    if _NC is None:
        _NC = _build_program()
    return _NC


def kernel(x, params):
    w, xcols = _prep_host(x, params)
    nc = _program()
    in_maps = []
    for c in range(NCORES):
        m = {k: v for k, v in w.items()}
        m["xcol"] = xcols[c]
        in_maps.append(m)
    r = run_bass_kernel_spmd(nc, in_maps, core_ids=list(range(NCORES)))
    return np.concatenate([r.results[c]["out"] for c in range(NCORES)], axis=0)


# revision 6
# speedup vs baseline: 1.6291x; 1.6291x over previous
"""Trainium2 Bass kernel for nn_Backbone_Net (ResNet-ish backbone + FC + projection).

Data-parallel over 8 NeuronCores: 256 samples/core, processed in 16 chunks of 16.
Layout: channel-major activations [C_part, batch, H, W]. BN folded into weights
on host. conv1 via host im2col (bf16, K=147). 3x3 convs = 9 shifted f32r matmuls
accumulating in PSUM over zero-padded canvases. FC in bf16 with streamed weights.
Final PE transpose + on-device stereographic projection.
"""
import numpy as np
import ml_dtypes
from contextlib import ExitStack

import concourse.bass as bass
from concourse import bacc
import concourse.mybir as mybir
import concourse.tile as tile
from concourse.bass_utils import run_bass_kernel_spmd
from concourse.masks import make_identity

F32 = mybir.dt.float32
F32R = mybir.dt.float32r
BF16 = mybir.dt.bfloat16
AF = mybir.ActivationFunctionType
ALU = mybir.AluOpType
BF = ml_dtypes.bfloat16

NCORES = 8
B_TOTAL = 2048
BPC = B_TOTAL // NCORES      # 256 samples per core
BC = 16                      # chunk size (samples)
NCHUNK = BPC // BC           # 16
BN_EPS = 1e-5

# bias pack column registry: name -> (col_start, ncols, rows)
BIAS_SPECS = (
    [("bn1", 64), ("l1b0_b1", 64), ("l1b0_b2", 64), ("l1b0_b3", 256)]
    + [(f"l1b{i}_b{j}", c) for i in (1, 2) for j, c in ((1, 64), (2, 64), (3, 256))]
    + [(f"l2b{i}_b{j}", c) for i in range(4) for j, c in ((1, 128), (2, 128), (3, 512))]
    + [("fc_b", 512), ("fc1_b", 256)]
)


def _bias_cols():
    cols = {}
    c = 0
    for name, ch in BIAS_SPECS:
        n = max(1, ch // 128)
        cols[name] = (c, n, min(ch, 128))
        c += n
    return cols, c


BCOLS, NBIAS = _bias_cols()


def _fold(w, bn):
    s = bn["g"] / np.sqrt(bn["v"] + BN_EPS)
    return (w * s[:, None, None, None]).astype(np.float32), (bn["b"] - bn["m"] * s).astype(np.float32)


def _np(t):
    if isinstance(t, dict):
        return {k: _np(v) for k, v in t.items()}
    if isinstance(t, list):
        return [_np(v) for v in t]
    return np.asarray(t, dtype=np.float32)


def _prep_host(x, params):
    """Fold BN, build matmul-ready weight arrays + im2col input. Returns
    (shared weight map, per-core xcol slices)."""
    p = _np(params)
    x = np.asarray(x, dtype=np.float32)
    w = {}

    # --- conv1: 7x7/2 pad3, im2col K=147 ---
    wc, bc1 = _fold(p["conv1_w"], p["bn1"])                     # [64,3,7,7]
    wcol = wc.transpose(1, 2, 3, 0).reshape(147, 64)             # rows (c,ky,kx)
    w["c1wa"] = wcol[:128].astype(BF)
    w["c1wb"] = wcol[128:].astype(BF)

    biases = np.zeros((128, NBIAS), np.float32)

    def put_bias(name, vec):
        c0, n, rows = BCOLS[name]
        if n == 1:
            biases[:rows, c0] = vec
        else:
            biases[:, c0:c0 + n] = vec.reshape(n, 128).T

    put_bias("bn1", bc1)

    # --- layer1 ---
    for i, blk in enumerate(p["layer1"]):
        w1, b1 = _fold(blk["w1"], blk["b1"])    # [64, cin,1,1]
        w2, b2 = _fold(blk["w2"], blk["b2"])    # [64,64,3,3]
        w3, b3 = _fold(blk["w3"], blk["b3"])    # [256,64,1,1]
        w1l = w1[:, :, 0, 0].T                  # [cin, 64]
        if i == 0:
            dw, bd = _fold(blk["dw"], blk["db"])  # [256,64,1,1]
            w["l1b0_w1"] = w1l.astype(BF)         # [64,64] (reads bf16 m0)
            w["l1b0_dw"] = dw[:, :, 0, 0].T.reshape(64, 2, 128).astype(BF)
            b3 = b3 + bd
        else:
            w[f"l1b{i}_w1"] = w1l.reshape(2, 128, 64).transpose(1, 0, 2).copy()  # [128,2,64]
        w[f"l1b{i}_w2"] = w2.transpose(1, 2, 3, 0).reshape(64, 9, 64).copy()
        w[f"l1b{i}_w3"] = w3[:, :, 0, 0].T.reshape(64, 2, 128).copy()
        put_bias(f"l1b{i}_b1", b1)
        put_bias(f"l1b{i}_b2", b2)
        put_bias(f"l1b{i}_b3", b3)

    # --- layer2 ---
    for i, blk in enumerate(p["layer2"]):
        w1, b1 = _fold(blk["w1"], blk["b1"])    # [128, cin,1,1]
        w2, b2 = _fold(blk["w2"], blk["b2"])    # [128,128,3,3]
        w3, b3 = _fold(blk["w3"], blk["b3"])    # [512,128,1,1]
        w1l = w1[:, :, 0, 0].T                  # [cin,128]
        kg = w1l.shape[0] // 128
        w[f"l2b{i}_w1"] = w1l.reshape(kg, 128, 128).transpose(1, 0, 2).copy()  # [128,kg,128]
        w[f"l2b{i}_w2"] = w2.transpose(1, 2, 3, 0).reshape(128, 9, 128).copy()
        w[f"l2b{i}_w3"] = w3[:, :, 0, 0].T.reshape(128, 4, 128).copy()
        if i == 0:
            dw, bd = _fold(blk["dw"], blk["db"])  # [512,256,1,1]
            w["l2b0_dw"] = dw[:, :, 0, 0].T.reshape(2, 128, 4, 128).transpose(1, 0, 2, 3).copy()
            b3 = b3 + bd
        put_bias(f"l2b{i}_b1", b1)
        put_bias(f"l2b{i}_b2", b2)
        put_bias(f"l2b{i}_b3", b3)

    # --- fc layers (bf16) ---
    put_bias("fc_b", p["fc_b"])
    put_bias("fc1_b", p["fc1_b"])
    # FCW[m,g,p,s,mo] = fc_w[m*128+mo, (g*128+p)*16+s]
    w["fcw"] = p["fc_w"].reshape(4, 128, 4, 128, 16).transpose(0, 2, 3, 4, 1).astype(BF)
    # FC1W[p,k,m2,mo] = fc1_w[m2*128+mo, k*128+p]
    w["fc1w"] = p["fc1_w"].reshape(2, 128, 4, 128).transpose(3, 2, 0, 1).astype(BF)
    w["biases"] = biases

    # --- input im2col (bf16): xcol[(c,ky,kx), b, (y,x)] ---
    xp = np.zeros((B_TOTAL, 3, 38, 38), np.float32)
    xp[:, :, 3:35, 3:35] = x
    win = np.lib.stride_tricks.sliding_window_view(xp, (7, 7), axis=(2, 3))
    win = win[:, :, ::2, ::2]                                   # [B,3,16,16,7,7]
    xcol = win.transpose(1, 4, 5, 0, 2, 3).reshape(147, B_TOTAL, 256).astype(BF)
    xcols = [np.ascontiguousarray(xcol[:, c * BPC:(c + 1) * BPC, :]) for c in range(NCORES)]
    return w, xcols


def _build_program():
    nc = bacc.Bacc("TRN2", target_bir_lowering=False, debug=False)

    # ---- DRAM tensors ----
    d = {}
    def din(name, shape, dt):
        d[name] = nc.dram_tensor(name, shape, dt, kind="ExternalInput")
        return d[name]

    din("xcol", [147, BPC, 256], BF16)
    din("c1wa", [128, 64], BF16)
    din("c1wb", [19, 64], BF16)
    din("biases", [128, NBIAS], F32)
    din("l1b0_w1", [64, 64], BF16)
    din("l1b0_dw", [64, 2, 128], BF16)
    for i in range(3):
        din(f"l1b{i}_w2", [64, 9, 64], F32R)
        din(f"l1b{i}_w3", [64, 2, 128], F32R)
        if i > 0:
            din(f"l1b{i}_w1", [128, 2, 64], F32R)
    for i in range(4):
        kg = 2 if i == 0 else 4
        din(f"l2b{i}_w1", [128, kg, 128], F32R)
        din(f"l2b{i}_w2", [128, 9, 128], F32R)
        din(f"l2b{i}_w3", [128, 4, 128], F32R)
    din("l2b0_dw", [128, 2, 4, 128], F32R)
    din("fcw", [4, 4, 128, 16, 128], BF16)
    din("fc1w", [128, 4, 2, 128], BF16)
    out = nc.dram_tensor("out", [BPC, 257], F32, kind="ExternalOutput")

    with ExitStack() as ctx:
        tc = ctx.enter_context(tile.TileContext(nc))
        wgt = ctx.enter_context(tc.tile_pool(name="wgt", bufs=1))
        psum = ctx.enter_context(tc.tile_pool(name="ps", bufs=8, space="PSUM"))

        # ---- resident weights ----
        s = {}
        _early = {"c1wa", "c1wb", "biases"}
        _deferred_wdma = []
        for name, shape, dt in [
            ("c1wa", [128, 64], BF16), ("c1wb", [19, 64], BF16),
            ("biases", [128, NBIAS], F32),
            ("l1b0_w1", [64, 64], BF16), ("l1b0_dw", [64, 2, 128], BF16),
            ("l1b0_w2", [64, 9, 64], F32R), ("l1b0_w3", [64, 2, 128], F32R),
            ("l1b1_w1", [128, 2, 64], F32R), ("l1b1_w2", [64, 9, 64], F32R), ("l1b1_w3", [64, 2, 128], F32R),
            ("l1b2_w1", [128, 2, 64], F32R), ("l1b2_w2", [64, 9, 64], F32R), ("l1b2_w3", [64, 2, 128], F32R),
            ("l2b0_w1", [128, 2, 128], F32R), ("l2b0_w2", [128, 9, 128], F32R),
            ("l2b0_w3", [128, 4, 128], F32R), ("l2b0_dw", [128, 2, 4, 128], F32R),
            ("l2b1_w1", [128, 4, 128], F32R), ("l2b1_w2", [128, 9, 128], F32R), ("l2b1_w3", [128, 4, 128], F32R),
            ("l2b2_w1", [128, 4, 128], F32R), ("l2b2_w2", [128, 9, 128], F32R), ("l2b2_w3", [128, 4, 128], F32R),
            ("l2b3_w1", [128, 4, 128], F32R), ("l2b3_w2", [128, 9, 128], F32R), ("l2b3_w3", [128, 4, 128], F32R),
            ("fc1w", [128, 4, 2, 128], BF16),
        ]:
            t = wgt.tile(shape, dt, tag=name)
            if name in _early:
                nc.sync.dma_start(out=t, in_=d[name].ap())
            else:
                _deferred_wdma.append((t, d[name].ap()))
            s[name] = t

        def bias_ap(name, g=0):
            c0, n, rows = BCOLS[name]
            assert g < n
            return s["biases"][0:rows, c0 + g:c0 + g + 1]

        # persistent accumulator for FC inputs: A5[p, m, s, b] bf16
        A5 = wgt.tile([128, 4, 16, BPC], BF16, tag="A5")
        A5v = A5.rearrange("p m s b -> p m b s")

        ident = wgt.tile([128, 128], F32, tag="ident")
        make_identity(nc, ident)

        # ---- persistent padded canvases (borders zeroed once) ----
        c1p = wgt.tile([64, BC, 17, 17], BF16, tag="c1p")
        nc.gpsimd.memset(c1p[:, :, 0, :], 0.0)
        nc.gpsimd.memset(c1p[:, :, 1:17, 0], 0.0)
        l1mid = wgt.tile([64, BC, 10, 10], F32R, tag="l1mid")
        l2b0mid = wgt.tile([128, BC, 10, 10], F32R, tag="l2b0mid")
        for cv in (l1mid, l2b0mid):
            nc.gpsimd.memset(cv[:, :, 0, :].bitcast(F32), 0.0)
            nc.gpsimd.memset(cv[:, :, 9, :].bitcast(F32), 0.0)
            nc.gpsimd.memset(cv[:, :, 1:9, 0].bitcast(F32), 0.0)
            nc.gpsimd.memset(cv[:, :, 1:9, 9].bitcast(F32), 0.0)
        l2mids = []
        for cvi in range(2):
            cv = wgt.tile([128, BC, 6, 6], F32R, tag=f"l2mid{cvi}")
            nc.gpsimd.memset(cv[:, :, 0, :].bitcast(F32), 0.0)
            nc.gpsimd.memset(cv[:, :, 5, :].bitcast(F32), 0.0)
            nc.gpsimd.memset(cv[:, :, 1:5, 0].bitcast(F32), 0.0)
            nc.gpsimd.memset(cv[:, :, 1:5, 5].bitcast(F32), 0.0)
            l2mids.append(cv)

        # ---- chunk-phase pools ----
        with ExitStack() as cctx:
            xap = cctx.enter_context(tc.tile_pool(name="xap", bufs=2))
            xbp = cctx.enter_context(tc.tile_pool(name="xbp", bufs=2))
            mpool = cctx.enter_context(tc.tile_pool(name="mpool", bufs=2))
            apool = cctx.enter_context(tc.tile_pool(name="apool", bufs=3))

            def stage_a(ci):
                """conv1 + maxpool for chunk ci, as emission closures.
                Returns (m0_tile, [pieces])."""
                b0 = ci * BC
                m0 = mpool.tile([64, BC, 8, 8], BF16, tag="m0")
                tcol = mpool.tile([64, BC, 17, 8], BF16, tag="tcol")
                st = {}

                def conv_piece(h, q):
                    def run():
                        if q == 0:
                            xa = xap.tile([128, 8 * 256], BF16, tag="xa")
                            xb = xbp.tile([19, 8 * 256], BF16, tag="xb")
                            nc.sync.dma_start(out=xa, in_=d["xcol"].ap()[0:128, b0 + 8 * h:b0 + 8 * h + 8, :])
                            nc.sync.dma_start(out=xb, in_=d["xcol"].ap()[128:147, b0 + 8 * h:b0 + 8 * h + 8, :])
                            st[h] = (xa, xb)
                        xa, xb = st[h]
                        pss = []
                        for n in (2 * q, 2 * q + 1):
                            ps = psum.tile([64, 512], F32, tag="ps")
                            nc.tensor.matmul(ps, s["c1wa"], xa[:, n * 512:(n + 1) * 512],
                                             start=True, stop=False)
                            pss.append(ps)
                        for i, n in enumerate((2 * q, 2 * q + 1)):
                            nc.tensor.matmul(pss[i], s["c1wb"], xb[:, n * 512:(n + 1) * 512],
                                             start=False, stop=True)
                        for i, n in enumerate((2 * q, 2 * q + 1)):
                            smp = 8 * h + 2 * n
                            nc.vector.tensor_scalar(
                                out=c1p[:, smp:smp + 2, 1:17, 1:17],
                                in0=pss[i].rearrange("p (b y x) -> p b y x", b=2, y=16),
                                scalar1=bias_ap("bn1"), scalar2=0.0, op0=ALU.add, op1=ALU.max)
                    return run

                def pool_col():
                    nc.vector.tensor_max(out=tcol, in0=c1p[:, :, :, 0:15:2], in1=c1p[:, :, :, 1:16:2])
                    nc.vector.tensor_max(out=tcol, in0=tcol, in1=c1p[:, :, :, 2:17:2])

                def pool_row():
                    nc.vector.tensor_max(out=m0, in0=tcol[:, :, 0:15:2, :], in1=tcol[:, :, 1:16:2, :])
                    nc.vector.tensor_max(out=m0, in0=m0, in1=tcol[:, :, 2:17:2, :])

                pieces = [conv_piece(0, 0), conv_piece(0, 1), conv_piece(1, 0),
                          conv_piece(1, 1), pool_col, pool_row]
                return m0, pieces

            def stage_b(ci, m0):
                """layer1 + layer2 for chunk ci as a list of block closures."""
                b0 = ci * BC
                N8 = BC * 64
                N4 = BC * 16
                m0f = m0.rearrange("p b y x -> p (b y x)")
                st = {"a": None, "a3": None}
                blocks = []

                def l1block(blk):
                    nm = f"l1b{blk}"

                    def run():
                        a_prev = st["a"]
                        # mid1 = relu(w1 @ in), weight-stationary over n-tiles
                        pss = []
                        for n in range(2):
                            ps = psum.tile([64, 512], F32, tag="ps")
                            pss.append(ps)
                        if blk == 0:
                            for n in range(2):
                                nc.tensor.matmul(pss[n], s["l1b0_w1"],
                                                 m0f[:, n * 512:(n + 1) * 512], start=True, stop=True)
                        else:
                            for g in range(2):
                                for n in range(2):
                                    nc.tensor.matmul(pss[n], s[f"{nm}_w1"][:, g, :],
                                                     a_prev[:, g, n * 512:(n + 1) * 512],
                                                     start=(g == 0), stop=(g == 1))
                        for n in range(2):
                            nc.scalar.activation(
                                out=l1mid[:, 8 * n:8 * n + 8, 1:9, 1:9],
                                in_=pss[n].rearrange("p (b y x) -> p b y x", b=8, y=8),
                                func=AF.Relu, bias=bias_ap(f"{nm}_b1"), scale=1.0)
                        # mid2 = relu(conv3x3(mid1)), tap-stationary over halves
                        mid2 = mpool.tile([64, BC, 8, 8], F32R, tag="mid2l1")
                        ps2 = []
                        for bh in range(2):
                            ps = psum.tile([64, 512], F32, tag="ps")
                            ps2.append(ps)
                        for t in range(9):
                            ky, kx = t // 3, t % 3
                            for bh in range(2):
                                nc.tensor.matmul(ps2[bh], s[f"{nm}_w2"][:, t, :],
                                                 l1mid[:, 8 * bh:8 * bh + 8, ky:ky + 8, kx:kx + 8],
                                                 start=(t == 0), stop=(t == 8))
                        for bh in range(2):
                            nc.scalar.activation(
                                out=mid2[:, 8 * bh:8 * bh + 8, :, :],
                                in_=ps2[bh].rearrange("p (b y x) -> p b y x", b=8, y=8),
                                func=AF.Relu, bias=bias_ap(f"{nm}_b2"), scale=1.0)
                        mid2f = mid2.rearrange("p b y x -> p (b y x)")
                        # out = relu(w3 @ mid2 (+ dw @ m0 | + identity))
                        a_new = apool.tile([128, 2, N8], F32R, tag="l1a", bufs=4)
                        for g in range(2):
                            ps3 = []
                            for n in range(2):
                                ps = psum.tile([128, 512], F32, tag="ps")
                                ps3.append(ps)
                                nc.tensor.matmul(ps, s[f"{nm}_w3"][:, g, :],
                                                 mid2f[:, n * 512:(n + 1) * 512],
                                                 start=True, stop=(blk != 0))
                            for n in range(2):
                                ns = slice(n * 512, (n + 1) * 512)
                                if blk == 0:
                                    nc.tensor.matmul(ps3[n], s["l1b0_dw"][:, g, :], m0f[:, ns],
                                                     start=False, stop=True)
                                    nc.scalar.activation(out=a_new[:, g, ns], in_=ps3[n],
                                                         func=AF.Relu,
                                                         bias=bias_ap(f"{nm}_b3", g), scale=1.0)
                                else:
                                    nc.vector.scalar_tensor_tensor(
                                        out=a_new[:, g, ns], in0=ps3[n],
                                        scalar=bias_ap(f"{nm}_b3", g),
                                        in1=a_prev[:, g, ns], op0=ALU.add, op1=ALU.add)
                        if blk != 0:
                            af = a_new.rearrange("p g n -> p (g n)")
                            nc.scalar.activation(out=af, in_=af, func=AF.Relu)
                        st["a"] = a_new
                        if blk == 2:
                            st["a3"] = a_new
                    return run

                def l2block(blk):
                    nm = f"l2b{blk}"

                    def run():
                        a_prev = st["a"]
                        a3 = st["a3"]
                        if blk == 0:
                            canvas = l2b0mid
                            ps1 = []
                            for n in range(2):
                                ps = psum.tile([128, 512], F32, tag="ps")
                                ps1.append(ps)
                            for g in range(2):
                                for n in range(2):
                                    nc.tensor.matmul(ps1[n], s[f"{nm}_w1"][:, g, :],
                                                     a3[:, g, n * 512:(n + 1) * 512],
                                                     start=(g == 0), stop=(g == 1))
                            for n in range(2):
                                nc.scalar.activation(
                                    out=canvas[:, 8 * n:8 * n + 8, 1:9, 1:9],
                                    in_=ps1[n].rearrange("p (b y x) -> p b y x", b=8, y=8),
                                    func=AF.Relu, bias=bias_ap(f"{nm}_b1"), scale=1.0)
                        else:
                            canvas = l2mids[blk % 2]
                            ps = psum.tile([128, 256], F32, tag="ps")
                            for g in range(4):
                                nc.tensor.matmul(ps, s[f"{nm}_w1"][:, g, :], a_prev[:, g, :],
                                                 start=(g == 0), stop=(g == 3))
                            nc.scalar.activation(
                                out=canvas[:, :, 1:5, 1:5],
                                in_=ps.rearrange("p (b y x) -> p b y x", b=BC, y=4),
                                func=AF.Relu, bias=bias_ap(f"{nm}_b1"), scale=1.0)
                        # mid2 = relu(conv3x3(canvas)), stride 2 for blk 0
                        mid2 = mpool.tile([128, BC, 4, 4], F32R, tag="mid2l2")
                        ps = psum.tile([128, 256], F32, tag="ps")
                        for t in range(9):
                            ky, kx = t // 3, t % 3
                            if blk == 0:
                                rhs = canvas[:, :, ky:ky + 8:2, kx:kx + 8:2]
                            else:
                                rhs = canvas[:, :, ky:ky + 4, kx:kx + 4]
                            nc.tensor.matmul(ps, s[f"{nm}_w2"][:, t, :], rhs,
                                             start=(t == 0), stop=(t == 8))
                        nc.vector.tensor_scalar(
                            out=mid2, in0=ps.rearrange("p (b y x) -> p b y x", b=BC, y=4),
                            scalar1=bias_ap(f"{nm}_b2"), scalar2=0.0, op0=ALU.add, op1=ALU.max)
                        mid2f = mid2.rearrange("p b y x -> p (b y x)")
                        # out = relu(w3 @ mid2 (+ dw @ a3 | + identity))
                        last = blk == 3
                        if not last:
                            a_new = apool.tile([128, 4, N4], F32R, tag="l2a")
                        a3s = a3.rearrange("p g (b y x) -> p g b y x", b=BC, y=8)
                        for m in range(4):
                            ps = psum.tile([128, 256], F32, tag="ps")
                            nc.tensor.matmul(ps, s[f"{nm}_w3"][:, m, :], mid2f,
                                             start=True, stop=(blk != 0))
                            if blk == 0:
                                for g in range(2):
                                    nc.tensor.matmul(ps, s["l2b0_dw"][:, g, m, :],
                                                     a3s[:, g, :, 0:8:2, 0:8:2],
                                                     start=False, stop=(g == 1))
                                nc.scalar.activation(out=a_new[:, m, :], in_=ps, func=AF.Relu,
                                                     bias=bias_ap(f"{nm}_b3", m), scale=1.0)
                            elif not last:
                                nc.vector.scalar_tensor_tensor(
                                    out=a_new[:, m, :], in0=ps,
                                    scalar=bias_ap(f"{nm}_b3", m),
                                    in1=a_prev[:, m, :], op0=ALU.add, op1=ALU.add)
                            else:
                                nc.vector.scalar_tensor_tensor(
                                    out=A5v[:, m, b0:b0 + BC, :],
                                    in0=ps.rearrange("p (b s) -> p b s", b=BC),
                                    scalar=bias_ap(f"{nm}_b3", m),
                                    in1=a_prev[:, m, :].rearrange("p (b s) -> p b s", b=BC),
                                    op0=ALU.add, op1=ALU.add)
                        if blk == 0:
                            pass
                        elif not last:
                            af = a_new.rearrange("p m n -> p (m n)")
                            nc.scalar.activation(out=af, in_=af, func=AF.Relu)
                        else:
                            av = A5v[:, :, b0:b0 + BC, :]
                            nc.scalar.activation(out=av, in_=av, func=AF.Relu)
                        st["a"] = a_new if not last else None
                    return run

                for blk in range(3):
                    blocks.append(l1block(blk))
                for blk in range(4):
                    blocks.append(l2block(blk))
                return blocks

            # software pipeline, 3 streams interleaved per step:
            #   conv1+maxpool(step) | layer1(step-1) | layer2(step-2)
            pend = {}
            for step in range(NCHUNK + 2):
                apieces = []
                if step < NCHUNK:
                    m0, apieces = stage_a(step)
                    pend[step] = stage_b(step, m0)
                b1 = pend[step - 1][:3] if (step - 1) in pend else []
                b2 = pend[step - 2][3:] if (step - 2) in pend else []
                srcs = [b2, apieces, b1]
                idx = [0, 0, 0]
                for who in (0, 1, 2, 1, 0, 1, 2, 1, 0, 1, 2, 1, 0):
                    if idx[who] < len(srcs[who]):
                        srcs[who][idx[who]]()
                        idx[who] += 1
                for who in range(3):
                    while idx[who] < len(srcs[who]):
                        srcs[who][idx[who]]()
                        idx[who] += 1
                if step == 0:
                    for wt, wap in _deferred_wdma:
                        nc.sync.dma_start(out=wt, in_=wap)

        # ---- FC phase ----
        with ExitStack() as fctx:
            fcp = fctx.enter_context(tc.tile_pool(name="fcp", bufs=2))
            fc1 = fctx.enter_context(tc.tile_pool(name="fc1", bufs=1))
            z1 = fc1.tile([128, 4, BPC], BF16, tag="z1")
            for m in range(4):
                fcw_t = fcp.tile([128, 4, 16, 128], BF16, tag="fcw")
                nc.sync.dma_start(out=fcw_t, in_=d["fcw"].ap()[m].rearrange("g p s mo -> p g s mo"))
                ps = psum.tile([128, BPC], F32, tag="ps")
                for g in range(4):
                    for si in range(16):
                        nc.tensor.matmul(ps, fcw_t[:, g, si, :], A5[:, g, si, :],
                                         start=(g == 0 and si == 0), stop=(g == 3 and si == 15))
                nc.scalar.activation(out=z1[:, m, :], in_=ps, func=AF.Relu,
                                     bias=bias_ap("fc_b", m), scale=1.0)
            y2 = fc1.tile([128, 2, BPC], F32, tag="y2")
            for m2 in range(2):
                ps = psum.tile([128, BPC], F32, tag="ps")
                for k in range(4):
                    nc.tensor.matmul(ps, s["fc1w"][:, k, m2, :], z1[:, k, :],
                                     start=(k == 0), stop=(k == 3))
                nc.scalar.activation(out=y2[:, m2, :], in_=ps, func=AF.Relu,
                                     bias=bias_ap("fc1_b", m2), scale=1.0)
            # transpose y2 -> y2t[b_part, bh, ch]
            y2t = fc1.tile([128, 2, 257], F32, tag="y2t")
            for g in range(2):
                for bh in range(2):
                    ps = psum.tile([128, 128], F32, tag="ps")
                    nc.tensor.transpose(ps, y2[:, g, bh * 128:(bh + 1) * 128], ident)
                    nc.vector.tensor_copy(out=y2t[:, bh, g * 128:(g + 1) * 128], in_=ps)
            # stereographic projection (per-sample, samples on partitions)
            sq = fc1.tile([128, 256], F32, tag="sq")
            ss = fc1.tile([128, 2], F32, tag="ss")
            rec = fc1.tile([128, 2], F32, tag="rec")
            s_t = fc1.tile([128, 2], F32, tag="s_t")
            oms = fc1.tile([128, 2], F32, tag="oms")
            for bh in range(2):
                nc.scalar.activation(out=sq, in_=y2t[:, bh, 0:256], func=AF.Square,
                                     accum_out=ss[:, bh:bh + 1])
            nc.vector.tensor_scalar(out=rec, in0=ss, scalar1=1.0, scalar2=None, op0=ALU.add)
            nc.vector.reciprocal(out=rec, in_=rec)
            nc.vector.scalar_tensor_tensor(out=s_t, in0=ss, scalar=-1.0, in1=rec,
                                           op0=ALU.add, op1=ALU.mult)
            nc.vector.tensor_scalar(out=oms, in0=s_t, scalar1=-1.0, scalar2=1.0,
                                    op0=ALU.mult, op1=ALU.add)
            for bh in range(2):
                nc.vector.tensor_scalar(out=y2t[:, bh, 0:256], in0=y2t[:, bh, 0:256],
                                        scalar1=oms[:, bh:bh + 1], scalar2=None, op0=ALU.mult)
                nc.vector.tensor_copy(out=y2t[:, bh, 256:257], in_=s_t[:, bh:bh + 1])
            nc.sync.dma_start(out=out.ap().rearrange("(bh p) c -> p bh c", p=128), in_=y2t)

    nc.compile()
    return nc


_NC = None


def _program():
    global _NC
    if _NC is None:
        _NC = _build_program()
    return _NC


def kernel(x, params):
    w, xcols = _prep_host(x, params)
    nc = _program()
    in_maps = []
    for c in range(NCORES):
        m = {k: v for k, v in w.items()}
        m["xcol"] = xcols[c]
        in_maps.append(m)
    r = run_bass_kernel_spmd(nc, in_maps, core_ids=list(range(NCORES)))
    return np.concatenate([r.results[c]["out"] for c in range(NCORES)], axis=0)
